# revision 10
# baseline (speedup 1.0000x reference)
"""Trainium2 Bass kernel for a causal single-head attention block.

Problem: y = softmax(mask(Q K^T / sqrt(H))) V with
  x  [B=4, T=4096, C=1024] f32,  Wq/Wk/Wv [C, H=64] f32.

Sharding (8 NeuronCores): data-parallel over B across core pairs;
within a pair, T is split by interleaved 512-row tiles (rank r owns
global q-tiles {2s+r}).  Each core projects K/V for its own 2048 rows,
the pair exchanges them via AllGather, and each core runs a
flash-attention style pair-of-kc outer loop over its own query rows.
The graph is identical on all 8 cores (SPMD); all rank-dependent
causality is delivered via input *data* (per-rank mask sheets).

Structure (v3):
 - xT streams in per-slot, split across the sync/scalar/gpsimd DMA
   queues so the first projection starts ~2us in.
 - K and V are projected together ([Wk|Wv] packed stationary); V^T is
   transposed to t-layout right after each slot so the K/V AllGathers
   can fire per half (slots 0-1 -> k-pairs 0-7, slots 2-3 -> 8-15).
 - Q is projected as [Wq|Wq] so both PE-array row-group halves get a
   copy of Q^T (S^T matmuls pair kc chunks on disjoint row groups so
   LDWEIGHTS overlaps the other half's matmul).
 - exp on ACT over [128,1024] PSUM tiles (scale folded); causal mask
   is a bf16 DVE multiply on the last 4 pairs of each slot's window.
 - Row-sums ride along as a ones-column in V (65-wide PV stationary);
   normalization gathers the 4 slot row-sums, reshapes via a DRAM
   bounce, and does one DVE reciprocal_approx_fast per half.
"""

import numpy as np
import ml_dtypes

import concourse.bass as bass
import concourse.bacc as bacc
import concourse.mybir as mybir
from concourse.tile import TileContext
from concourse.tile_rust import add_dep_helper
from concourse.bass_utils import run_bass_kernel_spmd

BF16 = mybir.dt.bfloat16
F32 = mybir.dt.float32
bf16 = ml_dtypes.bfloat16

B, T, C, H = 4, 4096, 1024, 64
N_CORES = 8
TOWN = 2048          # rows owned per core
NSLOT = 4            # q-tiles of 512 rows per core
QT = 512
NKC = 32             # global 128-row k-chunks
NPAIR = 16           # global 256-row k-pair chunks
CC_K = H * TOWN      # K^T shard elements
CC_V = 128 * 1024    # V shard elements (t-layout)


def build_bass(dbg=False):
    nc = bacc.Bacc(
        "TRN2",
        target_bir_lowering=False,
        debug=False,
        enable_asserts=False,
        num_devices=N_CORES,
    )

    if dbg:
        d_lrow = nc.declare_dram_parameter(
            "d_lrow", [1, NSLOT * QT], F32, isOutput=True
        )
        d_rrow = nc.declare_dram_parameter(
            "d_rrow", [1, NSLOT * QT], F32, isOutput=True
        )

    xT = nc.declare_dram_parameter("xT", [C, TOWN], BF16, isOutput=False)
    wkv = nc.declare_dram_parameter("wkv", [C, 128], BF16, isOutput=False)
    wqq = nc.declare_dram_parameter("wqq", [C, 128], BF16, isOutput=False)
    ident = nc.declare_dram_parameter("ident", [128, H], BF16, isOutput=False)
    mask = nc.declare_dram_parameter("mask", [128, 4 * 1024], BF16, isOutput=False)
    out = nc.declare_dram_parameter("out", [H, TOWN], F32, isOutput=True)

    cc_in_k = nc.dram_tensor("cc_in_k", [CC_K], BF16)
    cc_out_k = [nc.dram_tensor(f"cc_out_k{i}", [CC_K], BF16) for i in range(2)]
    cc_in_v = nc.dram_tensor("cc_in_v", [CC_V], BF16)
    cc_out_v = [nc.dram_tensor(f"cc_out_v{i}", [CC_V], BF16) for i in range(2)]
    lsc = [nc.dram_tensor(f"lsc{i}", [2 * QT], F32) for i in range(2)]
    rsc = [nc.dram_tensor(f"rsc{i}", [2 * QT], F32) for i in range(2)]
    groups = [[2 * i, 2 * i + 1] for i in range(N_CORES // 2)]

    with TileContext(nc) as tc:
        with (
            tc.tile_pool(name="persist", bufs=1) as pp,
            tc.tile_pool(name="work", bufs=3) as wp,
        ):
            # ---- persistent SBUF tensors ----
            xT_sb = pp.tile([128, 8, TOWN], BF16, tag="xT")
            wkv_sb = pp.tile([128, 8, 128], BF16, tag="wkv")
            wqq_sb = pp.tile([128, 8, 128], BF16, tag="wqq")
            id_sb = pp.tile([128, H], BF16, tag="ident")
            mask_sb = pp.tile([128, 4 * 1024], BF16, tag="mask")
            # K^T (rows 0:64) and V^T (rows 64:128) staging for own rows
            kv_stage = pp.tile([128, TOWN], BF16, tag="kvstage")
            vstage = pp.tile([128, 1024], BF16, tag="vstage")
            qT2 = [
                pp.tile([128, QT], BF16, tag=f"q{s}", name=f"qT2_{s}")
                for s in range(NSLOT)
            ]
            # K^T pair layout, split by half: pair p in half hf=p//8,
            # cols (p%8)*128..; chunk 2p at partitions 0:64, 2p+1 at 64:128
            kT2 = [
                pp.tile([128, 8 * 128], BF16, tag=f"kT2{i}", name=f"kT2_{i}")
                for i in range(2)
            ]
            # V t-layout chunks + ones column, split by half (16 chunks each)
            vaug = [
                pp.tile([128, 16, H + 1], BF16, tag=f"vaug{i}", name=f"vaug_{i}")
                for i in range(2)
            ]
            ones_sb = pp.tile([1, H], F32, tag="ones")
            dume = pp.tile([1, 8], F32, tag="dume")
            lrow = [
                pp.tile([1, 2 * QT], F32, tag=f"lrow{i}", name=f"lrow_{i}")
                for i in range(2)
            ]
            lsum_t = [
                pp.tile([128, 8], F32, tag=f"lsumt{i}", name=f"lsum_t{i}")
                for i in range(2)
            ]
            rec_t = [
                pp.tile([128, 8], F32, tag=f"rect{i}", name=f"rec_t{i}")
                for i in range(2)
            ]
            rec_row = [
                pp.tile([1, 2 * QT], F32, tag=f"recrow{i}", name=f"rec_row{i}")
                for i in range(2)
            ]

            # ---- loads; spread across the three DMA trigger queues ----
            nc.vector.memset(ones_sb[:], 1.0)
            for i in range(2):
                nc.vector.memset(vaug[i][:, :, H : H + 1], 1.0)
            # preload the exp table set while DMAs run
            nc.scalar.activation(
                dume[:], ones_sb[0:1, 0:8], mybir.ActivationFunctionType.Exp
            )
            nc.scalar.dma_start(
                out=wkv_sb[:], in_=wkv[:].rearrange("(cc p) m -> p cc m", p=128)
            )
            # xT slot s, cc half h -> alternate queues; slot 0 first
            xq = [nc.sync, nc.gpsimd]
            for s in range(NSLOT):
                sl = slice(s * QT, (s + 1) * QT)
                for h in range(2):
                    xq[h].dma_start(
                        out=xT_sb[:, 4 * h : 4 * h + 4, sl],
                        in_=xT[h * 512 : (h + 1) * 512, sl].rearrange(
                            "(cc p) t -> p cc t", p=128
                        ),
                    )
            nc.scalar.dma_start(
                out=wqq_sb[:], in_=wqq[:].rearrange("(cc p) m -> p cc m", p=128)
            )
            nc.scalar.dma_start(out=id_sb[:], in_=ident[:])
            nc.scalar.dma_start(out=mask_sb[:], in_=mask[:])

            # ---- projections ----
            with (
                tc.tile_pool(name="proj_ps", bufs=3, space="PSUM") as proj_ps,
                tc.tile_pool(name="vt_ps", bufs=2, space="PSUM") as vt_ps,
            ):
                # pass 1: K^T | V^T for own rows; transposes interleaved
                for s in range(NSLOT):
                    sl = slice(s * QT, (s + 1) * QT)
                    ps = proj_ps.tile([128, QT], F32, tag="proj")
                    for cc in range(8):
                        nc.tensor.matmul(
                            ps[:],
                            wkv_sb[:, cc, :],
                            xT_sb[:, cc, sl],
                            start=(cc == 0),
                            stop=(cc == 7),
                        )
                    nc.vector.tensor_copy(kv_stage[:, sl], ps[:])
                    # V^T -> V (t-layout) for this slot's 4 t-chunks
                    for c in range(4):
                        tcn = 4 * s + c
                        pst = vt_ps.tile([128, H], BF16, tag="vt")
                        nc.tensor.transpose(
                            pst[:],
                            kv_stage[64:128, tcn * 128 : (tcn + 1) * 128],
                            id_sb[64:128, :],
                        )
                        nc.vector.tensor_copy(
                            vstage[:, tcn * H : (tcn + 1) * H], pst[:]
                        )
                    if s % 2 == 1:
                        hf = s // 2
                        hsl = slice(hf * 1024, (hf + 1) * 1024)
                        nc.gpsimd.dma_start(
                            out=cc_in_k[hf * CC_K // 2 : (hf + 1) * CC_K // 2]
                            .rearrange("(p t) -> p t", p=H),
                            in_=kv_stage[0:H, hsl],
                        )
                        nc.gpsimd.collective_compute(
                            "AllGather",
                            mybir.AluOpType.bypass,
                            replica_groups=groups,
                            ins=[cc_in_k[hf * CC_K // 2 : (hf + 1) * CC_K // 2]],
                            outs=[cc_out_k[hf][:]],
                        )
                        nc.gpsimd.dma_start(
                            out=cc_in_v[hf * CC_V // 2 : (hf + 1) * CC_V // 2]
                            .rearrange("(p c) -> p c", p=128),
                            in_=vstage[:, hf * 512 : (hf + 1) * 512],
                        )
                        vcc = nc.gpsimd.collective_compute(
                            "AllGather",
                            mybir.AluOpType.bypass,
                            replica_groups=groups,
                            ins=[cc_in_v[hf * CC_V // 2 : (hf + 1) * CC_V // 2]],
                            outs=[cc_out_v[hf][:]],
                        )

                # pass 2: Q^T duplicated to both halves ([Wq|Wq] stationary)
                for s in range(NSLOT):
                    sl = slice(s * QT, (s + 1) * QT)
                    ps = proj_ps.tile([128, QT], F32, tag="proj")
                    for cc in range(8):
                        mmq = nc.tensor.matmul(
                            ps[:],
                            wqq_sb[:, cc, :],
                            xT_sb[:, cc, sl],
                            start=(cc == 0),
                            stop=(cc == 7),
                        )
                        if s == 0 and cc == 0:
                            # keep the scheduler from front-running Q-proj
                            # ahead of the K/V path that feeds the collectives
                            add_dep_helper(
                                mmq.ins, vcc.ins, sync=False, reason="q after cc"
                            )
                    nc.vector.tensor_copy(qT2[s][:], ps[:])

            # ---- readback of gathered K^T and V into compute layouts ----
            # AG half hf covers slots 0-1/2-3 of both ranks = tiles 4hf..4hf+3
            # shard-half layout: [gp(rank)][h, s2(2 slots), c(4), kk(128)]
            for hf in range(2):
                ck = cc_out_k[hf][:].rearrange("(gp h sc) -> gp h sc", gp=2, h=H)
                cv = cc_out_v[hf][:].rearrange("(gp p sc) -> gp p sc", gp=2, p=128)
                for gp in range(2):
                    for s2 in range(2):
                        g = 2 * (2 * hf + s2) + gp  # global tile
                        # local pair index within half: tile g pairs 2g,2g+1
                        lp = 2 * g - 8 * hf
                        ck_s = ck[gp, :, s2 * QT : (s2 + 1) * QT].rearrange(
                            "h (chalf hh kk) -> h chalf hh kk", chalf=2, hh=2
                        )
                        for hh in range(2):
                            nc.sync.dma_start(
                                out=kT2[hf][
                                    hh * 64 : (hh + 1) * 64,
                                    lp * 128 : (lp + 2) * 128,
                                ].rearrange("h (chalf kk) -> h chalf kk", chalf=2),
                                in_=ck_s[:, :, hh, :],
                            )
                        nc.scalar.dma_start(
                            out=vaug[hf][:, 4 * g - 16 * hf : 4 * g - 16 * hf + 4, 0:H],
                            in_=cv[gp, :, s2 * 256 : (s2 + 1) * 256].rearrange(
                                "p (c h) -> p c h", h=H
                            ),
                        )

            # ---- attention: pair-of-kc outer flash loop ----
            with (
                tc.tile_pool(name="swide", bufs=2, space="PSUM") as sp,
                tc.tile_pool(name="yacc", bufs=1, space="PSUM") as yp,
            ):
                y_acc = [
                    yp.tile([128, QT], F32, tag=f"y{s}", name=f"y_acc{s}")
                    for s in range(NSLOT)
                ]

                def norm_batch(bi):
                    # slots 2bi, 2bi+1 done: normalize and write out
                    nc.sync.dma_start(
                        out=lsc[bi][:].rearrange("(one f) -> one f", one=1),
                        in_=lrow[bi][:],
                    )
                    nc.sync.dma_start(
                        out=lsum_t[bi][:],
                        in_=lsc[bi][:].rearrange("(p f) -> p f", p=128),
                    )
                    nc.vector.reciprocal_approx_fast(rec_t[bi][:], lsum_t[bi][:])
                    nc.sync.dma_start(
                        out=rsc[bi][:].rearrange("(p f) -> p f", p=128),
                        in_=rec_t[bi][:],
                    )
                    nc.sync.dma_start(
                        out=rec_row[bi][:],
                        in_=rsc[bi][:].rearrange("(one f) -> one f", one=1),
                    )
                    for j in range(2):
                        s = 2 * bi + j
                        bc = sp.tile([H, QT], F32, tag="swide")
                        nc.tensor.matmul(
                            bc[:],
                            ones_sb[:],
                            rec_row[bi][0:1, j * QT : (j + 1) * QT],
                            start=True,
                            stop=True,
                        )
                        bc_sb = wp.tile([H, QT], F32, tag="bcsb")
                        nc.vector.tensor_copy(bc_sb[:], bc[:])
                        y_sb = wp.tile([H, QT], F32, tag="ysb")
                        nc.vector.tensor_mul(y_sb[:], y_acc[s][0:H, :], bc_sb[:])
                        nc.sync.dma_start(
                            out=out[:, s * QT : (s + 1) * QT], in_=y_sb[:]
                        )

                for p in range(NPAIR):
                    hf, lp = p // 8, p % 8
                    for s in range(p // 4, NSLOT):
                        sw = sp.tile([128, 1024], F32, tag="swide")
                        nc.tensor.matmul(
                            sw[:, 0:QT],
                            kT2[hf][0:64, lp * 128 : (lp + 1) * 128],
                            qT2[s][0:64, :],
                            start=True,
                            stop=True,
                        )
                        nc.tensor.matmul(
                            sw[:, QT:1024],
                            kT2[hf][64:128, lp * 128 : (lp + 1) * 128],
                            qT2[s][64:128, :],
                            start=True,
                            stop=True,
                        )
                        pt = wp.tile([128, 1024], BF16, tag="pt")
                        nc.scalar.activation(
                            pt[:],
                            sw[:],
                            mybir.ActivationFunctionType.Exp,
                            scale=float(H) ** -0.5,
                        )
                        pp_idx = p - 4 * s
                        if pp_idx >= 0:
                            nc.vector.tensor_mul(
                                pt[:],
                                pt[:],
                                mask_sb[:, pp_idx * 1024 : (pp_idx + 1) * 1024],
                            )
                        for half in range(2):
                            kc = (2 * p + half) % 16
                            nc.tensor.matmul(
                                y_acc[s][0 : H + 1, :],
                                vaug[hf][:, kc, :],
                                pt[:, half * QT : (half + 1) * QT],
                                start=(p == 0 and half == 0),
                                stop=(p == 4 * s + 3 and half == 1),
                            )
                        if p == 4 * s + 3:
                            nc.vector.tensor_copy(
                                lrow[s // 2][0:1, (s % 2) * QT : (s % 2 + 1) * QT],
                                y_acc[s][H : H + 1, :],
                            )
                    if p == 7:
                        norm_batch(0)
                norm_batch(1)

                if dbg:
                    for i in range(2):
                        nc.sync.dma_start(
                            out=d_lrow[0:1, i * 1024 : (i + 1) * 1024],
                            in_=lrow[i][:],
                        )
                        nc.sync.dma_start(
                            out=d_rrow[0:1, i * 1024 : (i + 1) * 1024],
                            in_=rec_row[i][:],
                        )

    nc.compile()
    return nc


_NC_CACHE = None


def _get_nc():
    global _NC_CACHE
    if _NC_CACHE is None:
        _NC_CACHE = build_bass()
    return _NC_CACHE


def _make_in_maps(x, Wq, Wk, Wv):
    ident = np.zeros((128, H), dtype=bf16)
    ident[64:128, :] = np.eye(H, dtype=bf16)
    wkv = np.concatenate([Wk, Wv], axis=1).astype(bf16)
    wqq = np.concatenate([Wq, Wq], axis=1).astype(bf16)
    # mask sheets [128, 4*1024]: pair-position pp in 0..3, halves of 512
    # keep iff k <= q: p <= f + 512*r - 256*pp - 128*half
    p_idx = np.arange(128)[:, None]
    masks = []
    for r in range(2):
        m = np.zeros((128, 4, 2, QT), dtype=bf16)
        for ppos in range(4):
            for half in range(2):
                f_idx = np.arange(QT)[None, :]
                keep = p_idx <= f_idx + 512 * r - 256 * ppos - 128 * half
                m[:, ppos, half, :] = keep.astype(bf16)
        masks.append(np.ascontiguousarray(m.reshape(128, 4096)))
    in_maps = []
    for c in range(N_CORES):
        b, r = divmod(c, 2)
        rows = np.concatenate(
            [x[b, (2 * s + r) * QT : (2 * s + r + 1) * QT] for s in range(NSLOT)]
        )
        xT_c = np.ascontiguousarray(rows.T).astype(bf16)
        in_maps.append(
            {
                "xT": xT_c,
                "wkv": wkv,
                "wqq": wqq,
                "ident": ident,
                "mask": masks[r],
            }
        )
    return in_maps


def _assemble(results):
    y = np.empty((B, T, H), dtype=np.float32)
    for c in range(N_CORES):
        b, r = divmod(c, 2)
        yt = np.asarray(results[c]["out"], dtype=np.float32).T  # [2048, 64]
        for s in range(NSLOT):
            g = 2 * s + r
            y[b, g * QT : (g + 1) * QT] = yt[s * QT : (s + 1) * QT]
    return y


def run(x, Wq, Wk, Wv, trace=False):
    nc = _get_nc()
    in_maps = _make_in_maps(
        np.asarray(x, np.float32),
        np.asarray(Wq, np.float32),
        np.asarray(Wk, np.float32),
        np.asarray(Wv, np.float32),
    )
    res = run_bass_kernel_spmd(nc, in_maps, core_ids=list(range(N_CORES)), trace=trace)
    return _assemble(res.results), res


def kernel(x, Wq, Wk, Wv):
    y, _ = run(x, Wq, Wk, Wv)
    return y
